# revision 1
# baseline (speedup 1.0000x reference)
"""CircleLayer (histogram angle binning) Trainium2 Bass kernel.

Full-input contract: kernel(**inputs) takes the complete arrays, shards the
batch dim across 8 NeuronCores (pure data parallel), runs one SPMD Bass
program, and gathers the full [B, P, 2*D] output.

Layout strategy (per core: 512 samples = 4 tiles of 128):
  - host prep ships only what the device needs: last-frame neighbor coords,
    a validity-mask plane (-100 offset folds invalid neighbors below bin 0),
    and f_resonance transposed to [n, b, 64] bf16.
  - phase A (no fres dependency) computes per tile, in [128 samples(part),
    128 neighbors(free)] tiles spread over DVE/ACT/Pool: geometry, angle bins
    via exact fp32 thresholds (reproduces the reference's fp32 divide + int32
    trunc semantics bit-exactly), onehot scaled by 1/n and PE-transposed in
    bf16 into ohT[n, p, b] plus dd3T[n, b, (dist, dir, 1)].  Phase A is
    emitted op-major (each step across all 4 tiles) so the four independent
    chains pipeline across engines; Sqrt is batched in the prologue so ACT
    needs only two activation-table loads.
  - phase B: as each fres tile lands, 2 matmuls per sample into one PSUM
    [67, 8] block: rows 0-63 = fres.T @ onehotS (resonance means), rows
    64-66 = [dist, dir, 1].T @ onehotS (mean dist/dir + bias gate).
  - f_scan = relu(stage[64:67].T @ [W0; W1; b_ce]) via K=3 matmuls.
"""

import numpy as np

B, N, T, D = 4096, 128, 20, 64
P = 8
NCORES = 8
BC = B // NCORES  # samples per core
TILE = 128
NT = BC // TILE  # tiles per core
SW = D + 3  # stage rows: 64 resonance + dist + dir + ones

PI32 = np.float32(np.pi)
TWOPI32 = np.float32(2.0 * np.pi)
C32 = np.float32((2.0 * np.pi) / P)  # bin width as the reference computes it

# packed const layout, per tile chunk: [2 (ego) + N (mask) + 2N (neiL)]
CHUNK_W = 2 + N + 2 * N
CONST_W = NT * CHUNK_W


def _bin_thresholds():
    """T[p] = smallest fp32 x >= 0 with int32(fp32(x / C32)) >= p.

    Comparing dir >= T[p] then reproduces the reference's
    (dir / C32).astype(int32) binning exactly (fp32 division is monotone).
    """
    thr = [np.float32(0.0)]
    for p in range(1, P + 1):
        x = np.float32(np.float32(p) * C32)
        while int(np.float32(x / C32)) >= p:
            x = np.nextafter(x, np.float32(-np.inf))
        while int(np.float32(x / C32)) < p:
            x = np.nextafter(x, np.float32(np.inf))
        thr.append(np.float32(x))
    return thr


THR = _bin_thresholds()

_prog_cache = {}


def _build_program():
    import concourse.bass as bass
    import concourse.tile as tile
    from concourse import bacc, mybir
    from concourse.masks import make_identity

    f32 = mybir.dt.float32
    bf16 = mybir.dt.bfloat16
    OP = mybir.AluOpType
    AF = mybir.ActivationFunctionType

    nc = bacc.Bacc(
        "TRN2",
        target_bir_lowering=False,
        debug=False,
        enable_asserts=False,
        num_devices=NCORES,
    )

    cons = nc.dram_tensor("cons", [TILE, CONST_W], f32, kind="ExternalInput").ap()
    fresd = nc.dram_tensor("fresd", [N, BC * D], bf16, kind="ExternalInput").ap()
    w2d = nc.dram_tensor("w2d", [3, D], bf16, kind="ExternalInput").ap()
    res_out = nc.dram_tensor("res_out", [D, NT * 1024], bf16, kind="ExternalOutput").ap()
    fscan_out = nc.dram_tensor("fscan_out", [TILE, NT * 512], bf16, kind="ExternalOutput").ap()

    with tile.TileContext(nc) as tc:
        with (
            tc.tile_pool(name="const", bufs=1) as constp,
            tc.tile_pool(name="work", bufs=1) as wk,
            tc.tile_pool(name="geo", bufs=2) as geo,
            tc.tile_pool(name="small", bufs=2) as small,
            tc.tile_pool(name="stg", bufs=2) as stgp,
            tc.tile_pool(name="tpsum", bufs=1, space="PSUM") as tpsum,
            tc.tile_pool(name="opsum", bufs=2, space="PSUM") as opsum,
            tc.tile_pool(name="fpsum", bufs=1, space="PSUM") as fpsum,
        ):
            identb = constp.tile([128, 128], bf16)
            make_identity(nc, identb[:])
            cons_sb = constp.tile([TILE, CONST_W], f32)
            for t in range(NT):
                nc.sync.dma_start(
                    out=cons_sb[:, t * CHUNK_W : (t + 1) * CHUNK_W],
                    in_=cons[:, t * CHUNK_W : (t + 1) * CHUNK_W],
                )
            w2_sb = constp.tile([SW, D], bf16)
            nc.sync.dma_start(out=w2_sb[D : D + 3, :], in_=w2d)

            def ego_ap(t):
                return cons_sb[:, t * CHUNK_W : t * CHUNK_W + 2]

            def mask_ap(t):
                return cons_sb[:, t * CHUNK_W + 2 : t * CHUNK_W + 2 + N]

            def neix_ap(t):
                o = t * CHUNK_W + 2 + N
                return cons_sb[:, o : o + N]

            def neiy_ap(t):
                o = t * CHUNK_W + 2 + 2 * N
                return cons_sb[:, o : o + N]

            fres_sbs = []
            for t in range(NT):
                fres_sb = wk.tile([N, TILE * D], bf16, name=f"fres{t}", tag=f"fres{t}")
                nc.sync.dma_start(
                    out=fres_sb[:],
                    in_=fresd[:, t * TILE * D : (t + 1) * TILE * D],
                )
                fres_sbs.append(fres_sb)

            def wt(nm, t, dtype=f32):
                return wk.tile([TILE, N], dtype, name=f"{nm}{t}", tag=f"{nm}{t}")

            # --- prologue: rel coords + dist for all tiles (batches the Sqrt
            # ops so the ACT engine needs only one sqrt-table load) ---
            V = {}
            for t in range(NT):
                relx = wt("relx", t)
                nc.vector.tensor_scalar(relx[:], neix_ap(t), ego_ap(t)[:, 0:1], None, OP.subtract)
                rely = wt("rely", t)
                nc.vector.tensor_scalar(rely[:], neiy_ap(t), ego_ap(t)[:, 1:2], None, OP.subtract)
                sqx = geo.tile([TILE, N], f32, tag="sqx")
                nc.vector.tensor_tensor(sqx[:], relx[:], relx[:], op=OP.mult)
                dist = wt("dist", t)
                nc.vector.tensor_tensor(dist[:], rely[:], rely[:], op=OP.mult)
                nc.vector.tensor_tensor(dist[:], dist[:], sqx[:], op=OP.add)
                nc.scalar.sqrt(dist[:], dist[:])
                V[("relx", t)] = relx
                V[("rely", t)] = rely
                V[("dist", t)] = dist

            # --- phase A, op-major across tiles ---
            # stage A1: abs / octant prep
            for t in range(NT):
                ax = wt("ax", t)
                nc.scalar.activation(ax[:], V[("relx", t)][:], AF.Abs)
                ay = wt("ay", t)
                nc.scalar.activation(ay[:], V[("rely", t)][:], AF.Abs)
                xlt = wt("xlt", t)
                nc.gpsimd.tensor_scalar(xlt[:], V[("rely", t)][:], 0.0, None, OP.is_lt)
                ylt = wt("ylt", t)
                nc.gpsimd.tensor_scalar(ylt[:], V[("relx", t)][:], 0.0, None, OP.is_lt)
                mn = wt("mn", t)
                nc.vector.tensor_tensor(mn[:], ax[:], ay[:], op=OP.min)
                mx = wt("mx", t)
                nc.vector.tensor_tensor(mx[:], ax[:], ay[:], op=OP.max)
                le = wt("le", t)
                nc.vector.tensor_tensor(le[:], ax[:], ay[:], op=OP.is_le)
                scr = geo.tile([TILE, N], f32, tag="scr")
                invmx = wt("invmx", t)
                nc.vector.reciprocal_approx_accurate(out=invmx[:], in_=mx[:], scratch=scr[:])
                # qr in-place onto mn
                nc.vector.tensor_tensor(mn[:], mn[:], invmx[:], op=OP.mult)
                V[("xlt", t)] = xlt
                V[("ylt", t)] = ylt
                V[("qr", t)] = mn
                V[("le", t)] = le

            # stage A2: arctan + quadrant reconstruction
            for t in range(NT):
                atr = wt("atr", t)
                nc.scalar.activation(atr[:], V[("qr", t)][:], AF.Arctan)
                u1 = wt("u1", t)   # becomes aq
                nc.gpsimd.tensor_scalar(u1[:], atr[:], -1.0, float(np.float32(np.pi / 2)), OP.mult, OP.add)
                dd = wt("dd", t)   # becomes m1
                nc.gpsimd.tensor_scalar(dd[:], atr[:], 2.0, float(np.float32(-np.pi / 2)), OP.mult, OP.add)
                nc.vector.tensor_tensor(dd[:], V[("le", t)][:], dd[:], op=OP.mult)
                nc.vector.tensor_tensor(u1[:], u1[:], dd[:], op=OP.add)  # aq
                t2 = wt("t2", t)   # becomes m2, then th
                nc.gpsimd.tensor_scalar(t2[:], u1[:], -2.0, float(PI32), OP.mult, OP.add)
                nc.vector.tensor_tensor(t2[:], V[("xlt", t)][:], t2[:], op=OP.mult)
                nc.vector.tensor_tensor(t2[:], u1[:], t2[:], op=OP.add)  # th
                t3 = wt("t3", t)   # becomes m3, then dirw
                nc.gpsimd.tensor_scalar(t3[:], t2[:], -2.0, float(TWOPI32), OP.mult, OP.add)
                nc.vector.tensor_tensor(t3[:], V[("ylt", t)][:], t3[:], op=OP.mult)
                nc.vector.tensor_tensor(t3[:], t2[:], t3[:], op=OP.add)  # dirw
                dirm = wt("dirm", t)
                nc.vector.tensor_tensor(dirm[:], t3[:], mask_ap(t), op=OP.add)
                V[("dirw", t)] = t3
                V[("dirm", t)] = dirm

            # stage A3+A4 per tile, so each tile's ohT completes early
            for t in range(NT):
                dirm = V[("dirm", t)]
                ges = []
                for p in range(P + 1):
                    gep = wt(f"ge{p}_", t)
                    nc.gpsimd.tensor_scalar(gep[:], dirm[:], float(THR[p]), None, OP.is_ge)
                    ges.append(gep)
                nvec = small.tile([TILE, P], f32, tag="nvec")
                for p in range(P):
                    # oh_p in-place onto ge_p (ge_p dead afterwards)
                    nc.vector.scalar_tensor_tensor(
                        out=ges[p][:], in0=ges[p][:], scalar=0.0, in1=ges[p + 1][:],
                        op0=OP.add, op1=OP.subtract,
                        accum_out=nvec[:, p : p + 1],
                    )
                nadj = small.tile([TILE, P], f32, tag="nadj")
                nc.vector.tensor_scalar(nadj[:], nvec[:], 1e-4, None, OP.add)
                invn = small.tile([TILE, P], f32, tag="invn")
                nc.vector.reciprocal(invn[:], nadj[:])
                V[("ohs", t)] = ges[:P]

                tpAB = tpsum.tile([128, (P + 2) * TILE], bf16, tag="tpAB")
                for p in range(P):
                    ohb = wt(f"ohb{p}_", t, bf16)
                    nc.scalar.mul(ohb[:], V[("ohs", t)][p][:], invn[:, p : p + 1])
                    nc.tensor.transpose(tpAB[:, p * TILE : (p + 1) * TILE], ohb[:], identb[:])
                db = wt("db", t, bf16)
                nc.scalar.copy(db[:], V[("dist", t)][:])
                rb = wt("rb", t, bf16)
                nc.scalar.copy(rb[:], V[("dirw", t)][:])
                nc.tensor.transpose(tpAB[:, P * TILE : (P + 1) * TILE], db[:], identb[:])
                nc.tensor.transpose(tpAB[:, (P + 1) * TILE : (P + 2) * TILE], rb[:], identb[:])

                ohT = wk.tile([N, P * TILE], bf16, name=f"ohT{t}", tag=f"ohT{t}")
                nc.scalar.copy(ohT[:], tpAB[:, 0 : P * TILE])
                dd3T = wk.tile([N, TILE * 3], bf16, name=f"dd3T{t}", tag=f"dd3T{t}")
                dd3T_v = dd3T[:].rearrange("n (b c) -> n b c", c=3)
                nc.scalar.copy(dd3T_v[:, :, 0], tpAB[:, P * TILE : (P + 1) * TILE])
                nc.scalar.copy(dd3T_v[:, :, 1], tpAB[:, (P + 1) * TILE : (P + 2) * TILE])
                nc.gpsimd.memset(dd3T_v[:, :, 2], 1.0)
                V[("ohT", t)] = ohT
                V[("dd3T", t)] = dd3T

            # --- phase B: per-sample binning matmuls as fres tiles land ---
            for t in range(NT):
                fres_sb = fres_sbs[t]
                ohT_v = V[("ohT", t)][:].rearrange("n (p b) -> n p b", b=TILE)
                dd3T = V[("dd3T", t)]

                stage = stgp.tile([SW, 1024], bf16, tag="stage")
                for h in range(2):
                    pres = opsum.tile([SW, 512], f32, tag=f"pres{h}")
                    for s64 in range(64):
                        s = h * 64 + s64
                        nc.tensor.matmul(
                            pres[0:D, s64 * P : (s64 + 1) * P],
                            fres_sb[:, s * D : (s + 1) * D],
                            ohT_v[:, :, s],
                            start=True,
                            stop=True,
                        )
                        nc.tensor.matmul(
                            pres[D : D + 3, s64 * P : (s64 + 1) * P],
                            dd3T[:, s * 3 : (s + 1) * 3],
                            ohT_v[:, :, s],
                            start=True,
                            stop=True,
                        )
                    nc.scalar.copy(stage[:, h * 512 : (h + 1) * 512], pres[:])
                nc.sync.dma_start(
                    out=res_out[:, t * 1024 : (t + 1) * 1024],
                    in_=stage[0:D, :],
                )

                # --- f_scan = relu([mdist; mdir; 1].T @ [W0; W1; b]) ---
                fps = fpsum.tile([128, 512], f32, tag="fps")
                for c in range(8):
                    nc.tensor.matmul(
                        fps[:, c * D : (c + 1) * D],
                        stage[D : D + 3, c * TILE : (c + 1) * TILE],
                        w2_sb[D : D + 3, :],
                        start=True,
                        stop=True,
                    )
                fscan_sb = geo.tile([128, 512], bf16, tag="fscan")
                nc.scalar.activation(fscan_sb[:], fps[:], AF.Relu)
                nc.sync.dma_start(
                    out=fscan_out[:, t * 512 : (t + 1) * 512], in_=fscan_sb[:]
                )

    nc.compile()
    return nc


def _get_program():
    if "nc" not in _prog_cache:
        _prog_cache["nc"] = _build_program()
    return _prog_cache["nc"]


def _bf16():
    from concourse import mybir

    return np.dtype(mybir.dt.np(mybir.dt.bfloat16))


def prep_core_inputs(ego_traj_2d, nei_traj_2d, f_resonance, W_ce, b_ce):
    """Host-side layout: returns the per-core input maps."""
    bf16 = _bf16()
    ego_last = ego_traj_2d[:, -1, :]  # [B, 2]
    nei_last = nei_traj_2d[:, :, -1, :]  # [B, N, 2]
    # validity: all-zero padded trajectories are invalid -> -100 mask offset
    traj_sum = nei_traj_2d.reshape(B, N, T * 2).sum(axis=2)
    maskm_full = np.where(traj_sum == 0.0, np.float32(-100.0), np.float32(0.0))

    w2 = np.concatenate([W_ce, b_ce[None, :]], axis=0).astype(bf16)  # [3, D]

    in_maps = []
    for c in range(NCORES):
        rows = slice(c * BC, (c + 1) * BC)
        ego_c = ego_last[rows].reshape(NT, TILE, 2).transpose(1, 0, 2)
        msk_c = maskm_full[rows].reshape(NT, TILE, N).transpose(1, 0, 2)
        nei_c = nei_last[rows].reshape(NT, TILE, N, 2).transpose(1, 0, 3, 2)
        cons_c = np.empty((TILE, NT, CHUNK_W), dtype=np.float32)
        cons_c[:, :, 0:2] = ego_c
        cons_c[:, :, 2 : 2 + N] = msk_c
        cons_c[:, :, 2 + N :] = nei_c.reshape(TILE, NT, 2 * N)
        fres_c = np.ascontiguousarray(f_resonance[rows].transpose(1, 0, 2)).astype(bf16)
        in_maps.append(
            {
                "cons": cons_c.reshape(TILE, CONST_W),
                "fresd": fres_c.reshape(N, BC * D),
                "w2d": w2,
            }
        )
    return in_maps


def decode_core(res_raw, fscan_raw):
    """res_out[d, (t, h, s64, p)] and fscan_out[(s16, p), (t, hc, d)] -> [BC, P, 2D]."""
    r = (
        np.asarray(res_raw)
        .astype(np.float32)
        .reshape(D, NT, 2, 64, P)
        .transpose(1, 2, 3, 4, 0)
        .reshape(BC, P, D)
    )
    f = (
        np.asarray(fscan_raw)
        .astype(np.float32)
        .reshape(16, P, NT, 8, D)
        .transpose(2, 3, 0, 1, 4)
        .reshape(BC, P, D)
    )
    return np.concatenate([r, f], axis=-1)


def kernel(ego_traj_2d, nei_traj_2d, f_resonance, W_ce, b_ce):
    from concourse import bass_utils

    ego_traj_2d = np.asarray(ego_traj_2d, dtype=np.float32)
    nei_traj_2d = np.asarray(nei_traj_2d, dtype=np.float32)
    f_resonance = np.asarray(f_resonance, dtype=np.float32)
    W_ce = np.asarray(W_ce, dtype=np.float32)
    b_ce = np.asarray(b_ce, dtype=np.float32)

    nc = _get_program()
    in_maps = prep_core_inputs(ego_traj_2d, nei_traj_2d, f_resonance, W_ce, b_ce)

    res = bass_utils.run_bass_kernel_spmd(nc, in_maps, core_ids=list(range(NCORES)))
    outs = [
        decode_core(res.results[c]["res_out"], res.results[c]["fscan_out"])
        for c in range(NCORES)
    ]
    return np.concatenate(outs, axis=0)

